# revision 6
# baseline (speedup 1.0000x reference)
"""Trainium2 Bass kernel: 3D-window sparse multi-head attention.

Full op: out = SDPA(hid@Wq, hid@Wk, hid@Wv; 3D local window mask) @ Wo + bo
Shapes: hid [1, 2048, 1024], 16 heads x 64, grid (8 frames, 16, 16), window (3, 5, 5).

Sharding: head-parallel. Each of the 8 cores computes 2 heads end-to-end
(QKV projection slices, windowed attention, Wo row-slice projection) and
writes a full-shape fp32 partial; the host sums the 8 partials and adds bo.

On-chip layout (per core):
  hidT  [128, 8, 2048] fp16   hidden^T, D on partitions (8 chunks of 128)
  qT,kT [128, 2048]    fp16   rows = (head, head_dim), from lhsT=W slices
  v1    [128, 16, 66]  fp16   v in [s, hd] layout + ones column (denominator)
  scoresT blocks [kv=128, q=256] per (head, frame, kv-chunk); softmax without
  max-subtraction (scores are O(5)); window mask applied multiplicatively
  after exp; denominator = ones-row of the PV matmul; normalization via a
  K=128 broadcast matmul (emat) + elementwise multiply.
"""

import numpy as np

import concourse.bass as bass
import concourse.mybir as mybir
import concourse.tile as tile
from concourse import bacc
from concourse.bass import ds, ts
from concourse.bass_utils import run_bass_kernel_spmd

S, D, NH, HD = 2048, 1024, 16, 64
NCORES = 8
HPC = NH // NCORES          # heads per core = 2
F, GH, GW = 8, 16, 16       # frames, height, width (S = F*GH*GW)
WF, WH, WW = 3, 5, 5        # window sizes
T = GH * GW                 # tokens per frame = 256
P = 128
KC = D // P                 # 8 contraction chunks
SC = S // P                 # 16 seq chunks of 128
NQ = S // 512               # 4 free chunks of 512
F16 = mybir.dt.float16
F32 = mybir.dt.float32
EXP = mybir.ActivationFunctionType.Exp
MUL = mybir.AluOpType.mult

_nc_cache = {}


def build_nc(debug=False):
    key = bool(debug)
    if key in _nc_cache:
        return _nc_cache[key]
    nc = bacc.Bacc(None, target_bir_lowering=False, debug=False)

    hidt = nc.dram_tensor("hidt", [P, KC, S], F16, kind="ExternalInput")
    wq = nc.dram_tensor("wq", [P, KC, P], F16, kind="ExternalInput")
    wk = nc.dram_tensor("wk", [P, KC, P], F16, kind="ExternalInput")
    wv = nc.dram_tensor("wv", [P, KC, P], F16, kind="ExternalInput")
    wo = nc.dram_tensor("wo", [P, D], F16, kind="ExternalInput")
    m01 = nc.dram_tensor("m01", [P, 2, T], F16, kind="ExternalInput")
    emat = nc.dram_tensor("emat", [P, P], F16, kind="ExternalInput")
    out = nc.dram_tensor("out", [SC, P, D], F32, kind="ExternalOutput")
    dbg = {}
    if debug:
        dbg["qT"] = nc.dram_tensor("dbg_qt", [P, S], F16, kind="ExternalOutput")
        dbg["kT"] = nc.dram_tensor("dbg_kt", [P, S], F16, kind="ExternalOutput")
        dbg["v0"] = nc.dram_tensor("dbg_v0", [P, SC, 66], F16, kind="ExternalOutput")
        dbg["v1"] = nc.dram_tensor("dbg_v1", [P, SC, 66], F16, kind="ExternalOutput")
        dbg["oT"] = nc.dram_tensor("dbg_ot", [P, S], F32, kind="ExternalOutput")
        dbg["den"] = nc.dram_tensor("dbg_den", [33, S], F32, kind="ExternalOutput")
        dbg["oTn"] = nc.dram_tensor("dbg_otn", [P, S], F16, kind="ExternalOutput")

    with tile.TileContext(nc) as tc:
        with (
            tc.tile_pool(name="const", bufs=1) as cpool,
            tc.tile_pool(name="qk", bufs=1) as qkpool,
            tc.tile_pool(name="vp", bufs=1) as vpool,
            tc.tile_pool(name="attn", bufs=4) as apool,
            tc.tile_pool(name="acc", bufs=1) as accpool,
            tc.tile_pool(name="ostage", bufs=4) as opool,
        ):
            # ---- constant loads ----
            hidT_sb = cpool.tile([P, KC, S], F16, tag="hidT")
            for kc in range(KC):
                nc.sync.dma_start(hidT_sb[:, kc, :], hidt[:, kc, :])
            wq_sb = cpool.tile([P, KC, P], F16, tag="wq")
            nc.sync.dma_start(wq_sb[:], wq[:])
            wk_sb = cpool.tile([P, KC, P], F16, tag="wk")
            nc.sync.dma_start(wk_sb[:], wk[:])
            wv_sb = cpool.tile([P, KC, P], F16, tag="wv")
            nc.sync.dma_start(wv_sb[:], wv[:])
            wo_sb = cpool.tile([P, D], F16, tag="wo")
            nc.sync.dma_start(wo_sb[:], wo[:])
            m01_sb = cpool.tile([P, 2, T], F16, tag="m01")
            nc.sync.dma_start(m01_sb[:], m01[:])
            emat_sb = cpool.tile([P, P], F16, tag="emat")
            nc.sync.dma_start(emat_sb[:], emat[:])

            qT_sb = qkpool.tile([P, S], F16, tag="qT")
            kT_sb = qkpool.tile([P, S], F16, tag="kT")
            v1h0 = vpool.tile([P, SC, 66], F16, tag="v1h0")
            v1h1 = vpool.tile([P, SC, 66], F16, tag="v1h1")
            v1 = [v1h0, v1h1]
            oT_sb = accpool.tile([P, S], F32, tag="oT")
            oTn_sb = accpool.tile([P, S], F16, tag="oTn")
            den_sb = accpool.tile([33, S], F32, tag="den")
            denr_sb = accpool.tile([33, S], F32, tag="denr")
            rp_sb = accpool.tile([P, S], F16, tag="rp")

            # ones columns for the PV denominator trick (cols 64:66 so the
            # memset region is 4 bytes / one uint32 per partition-row)
            nc.vector.memset(v1h0[:, :, HD : HD + 2], 1.0)
            nc.vector.memset(v1h1[:, :, HD : HD + 2], 1.0)

            # ---- phase 1: QKV projections ----
            with (
                tc.tile_pool(name="pqk", bufs=4, space="PSUM") as pqk,
                tc.tile_pool(name="pv", bufs=2, space="PSUM") as pvp,
            ):
                for nch in range(NQ):
                    psq = pqk.tile([P, 512], F32, tag="pqk")
                    for kc in range(KC):
                        nc.tensor.matmul(
                            psq[:],
                            wq_sb[:, kc, :],
                            hidT_sb[:, kc, ts(nch, 512)],
                            start=(kc == 0),
                            stop=(kc == KC - 1),
                        )
                    nc.vector.tensor_copy(qT_sb[:, ts(nch, 512)], psq[:])
                    psk = pqk.tile([P, 512], F32, tag="pqk")
                    for kc in range(KC):
                        nc.tensor.matmul(
                            psk[:],
                            wk_sb[:, kc, :],
                            hidT_sb[:, kc, ts(nch, 512)],
                            start=(kc == 0),
                            stop=(kc == KC - 1),
                        )
                    nc.vector.tensor_copy(kT_sb[:, ts(nch, 512)], psk[:])
                for sc in range(SC):
                    psv = pvp.tile([P, P], F32, tag="psv")
                    for kc in range(KC):
                        nc.tensor.matmul(
                            psv[:],
                            hidT_sb[:, kc, ts(sc, P)],
                            wv_sb[:, kc, :],
                            start=(kc == 0),
                            stop=(kc == KC - 1),
                        )
                    nc.vector.tensor_copy(v1h0[:, sc, 0:HD], psv[:, 0:HD])
                    nc.vector.tensor_copy(v1h1[:, sc, 0:HD], psv[:, HD : 2 * HD])

            # ---- phase 2: windowed attention ----
            with (
                tc.tile_pool(name="pss", bufs=4, space="PSUM") as pssp,
                tc.tile_pool(name="pso", bufs=2, space="PSUM") as psop,
            ):
                for h in range(HPC):
                    hr = ds(h * HD, HD)
                    for f in range(F):
                        lo, hi = max(0, f - 1), min(F - 1, f + 1)
                        scs = list(range(2 * lo, 2 * hi + 2))
                        pso = psop.tile([P, T], F32, tag="pso")
                        for i, sckv in enumerate(scs):
                            pss = pssp.tile([P, T], F32, tag="pss")
                            nc.tensor.matmul(
                                pss[:],
                                kT_sb[hr, ds(sckv * P, P)],
                                qT_sb[hr, ds(f * T, T)],
                                start=True,
                                stop=True,
                            )
                            et = apool.tile([P, T], F16, tag="et")
                            nc.scalar.activation(et[:], pss[:], EXP)
                            pm = apool.tile([P, T], F16, tag="pm")
                            nc.gpsimd.tensor_tensor(
                                pm[:], et[:], m01_sb[:, sckv % 2, :], MUL
                            )
                            nc.tensor.matmul(
                                pso[0 : HD + 1, :],
                                v1[h][:, sckv, 0 : HD + 1],
                                pm[:],
                                start=(i == 0),
                                stop=(i == len(scs) - 1),
                            )
                        nc.vector.tensor_copy(
                            oT_sb[ds(h * HD, HD), ds(f * T, T)], pso[0:HD, :]
                        )
                        # engine APs must start at a 32-partition boundary:
                        # head h's denominator lives on partition 32*h
                        nc.vector.tensor_copy(
                            den_sb[32 * h : 32 * h + 1, ds(f * T, T)],
                            pso[HD : HD + 1, :],
                        )

            # ---- phase 3: normalization ----
            with tc.tile_pool(name="pbb", bufs=2, space="PSUM") as pbbp:
                nc.vector.memset(rp_sb[:], 0.0)
                for h in range(HPC):
                    r = 32 * h
                    nc.vector.reciprocal(
                        denr_sb[r : r + 1, :], den_sb[r : r + 1, :]
                    )
                    nc.vector.tensor_copy(
                        rp_sb[r : r + 1, :], denr_sb[r : r + 1, :]
                    )
                for nch in range(NQ):
                    pbb = pbbp.tile([P, 512], F32, tag="pbb")
                    nc.tensor.matmul(
                        pbb[:], emat_sb[:], rp_sb[:, ts(nch, 512)],
                        start=True, stop=True,
                    )
                    nc.vector.tensor_tensor(
                        oTn_sb[:, ts(nch, 512)], oT_sb[:, ts(nch, 512)], pbb[:], MUL
                    )

            # ---- phase 4: output projection ----
            with tc.tile_pool(name="pO", bufs=4, space="PSUM") as pOp:
                for sc in range(SC):
                    for n2 in range(2):
                        pO = pOp.tile([P, 512], F32, tag="pO")
                        nc.tensor.matmul(
                            pO[:],
                            oTn_sb[:, ts(sc, P)],
                            wo_sb[:, ts(n2, 512)],
                            start=True,
                            stop=True,
                        )
                        ob = opool.tile([P, 512], F32, tag="ob")
                        if sc % 2 == 0:
                            nc.vector.tensor_copy(ob[:], pO[:])
                        else:
                            nc.scalar.copy(ob[:], pO[:])
                        nc.sync.dma_start(out[sc, :, ts(n2, 512)], ob[:])

            if debug:
                nc.sync.dma_start(dbg["qT"][:], qT_sb[:])
                nc.sync.dma_start(dbg["kT"][:], kT_sb[:])
                nc.sync.dma_start(dbg["v0"][:], v1h0[:])
                nc.sync.dma_start(dbg["v1"][:], v1h1[:])
                nc.sync.dma_start(dbg["oT"][:], oT_sb[:])
                nc.sync.dma_start(dbg["den"][:], den_sb[:])
                nc.sync.dma_start(dbg["oTn"][:], oTn_sb[:])

    nc.compile()
    _nc_cache[key] = nc
    return nc


def make_in_maps(hidden_states, Wq, Wk, Wv, Wo):
    """Host-side shard + repack of full inputs into per-core input maps."""
    hid = np.asarray(hidden_states, np.float32).reshape(S, D)
    # hidT packed [ki, ko, s] with d = ko*128 + ki
    hidT_pk = np.ascontiguousarray(
        hid.T.reshape(KC, P, S).transpose(1, 0, 2)
    ).astype(np.float16)

    scale = 1.0 / np.sqrt(HD)
    Wq_s = np.asarray(Wq, np.float32) * scale
    Wk_ = np.asarray(Wk, np.float32)
    Wv_ = np.asarray(Wv, np.float32)
    Wo_ = np.asarray(Wo, np.float32)

    def pack_w(W, c):
        Wc = W[:, c * HPC * HD : (c + 1) * HPC * HD]  # [D, 128]
        return np.ascontiguousarray(
            Wc.reshape(KC, P, HPC * HD).transpose(1, 0, 2)
        ).astype(np.float16)

    # (h, w) window mask, 0/1, [256, 256] (symmetric) packed to [p, c, q]
    idx = np.arange(T)
    hh, ww = idx // GW, idx % GW
    m = (np.abs(hh[:, None] - hh[None, :]) <= WH // 2) & (
        np.abs(ww[:, None] - ww[None, :]) <= WW // 2
    )
    m01_pk = np.ascontiguousarray(
        m.astype(np.float16).reshape(2, P, T).transpose(1, 0, 2)
    )

    # head h's reciprocal denominator sits on partition 32*h of rp_sb
    emat = np.zeros((P, P), np.float16)
    emat[0, 0:HD] = 1.0
    emat[32, HD : 2 * HD] = 1.0

    in_maps = []
    for c in range(NCORES):
        in_maps.append(
            dict(
                hidt=hidT_pk,
                wq=pack_w(Wq_s, c),
                wk=pack_w(Wk_, c),
                wv=pack_w(Wv_, c),
                wo=Wo_[c * HPC * HD : (c + 1) * HPC * HD, :].astype(np.float16),
                m01=m01_pk,
                emat=emat,
            )
        )
    return in_maps


def kernel(
    hidden_states,
    Wq,
    Wk,
    Wv,
    Wo,
    bo,
    frames=F,
    height=GH,
    width=GW,
    wf=WF,
    wh=WH,
    ww=WW,
):
    assert (int(frames), int(height), int(width)) == (F, GH, GW)
    assert (int(wf), int(wh), int(ww)) == (WF, WH, WW)
    in_maps = make_in_maps(hidden_states, Wq, Wk, Wv, Wo)
    nc = build_nc(debug=False)
    res = run_bass_kernel_spmd(nc, in_maps, core_ids=list(range(NCORES)))
    acc = np.zeros((S, D), np.float32)
    for r in res.results:
        acc += r["out"].reshape(S, D)
    acc += np.asarray(bo, np.float32)[None, :]
    return acc.reshape(1, S, D)
